# revision 14
# baseline (speedup 1.0000x reference)
"""Fused causal multi-head attention block on 8 Trainium2 NeuronCores.

Problem (GPT-2 style attention, B=2, S=2048, D=1024, H=16, hd=64):
    qkv = x @ w_attn + b_attn ; split q,k,v ; per-head causal softmax(q k^T / 8) v
    out = attn_out @ w_proj + b_proj

Sharding: data parallel on batch (2) x tensor parallel on heads (4 groups of 4
heads). Core c -> batch c//4, head group c%4. Each core computes a partial
[S, D] output (its heads' slice of w_proj rows); host sums the 4 partials per
batch and adds b_proj.

Per-core kernel layout tricks:
- scores are computed TRANSPOSED (scoresT[key, query]) so the softmax
  denominator falls out of the attn@v matmul by appending a ones-column to v:
  [v | 1]^T @ exp(scoresT) yields the unnormalized output and the per-query
  denominator in one PSUM accumulation.
- matmul inputs are fp16 (full PE rate + fast weight loads); all accumulation
  is fp32 in PSUM. exp(s/8) is in [0, ~13], well inside fp16 range.
- causal masking: fully-masked blocks are skipped via restricted matmul
  widths; diagonal blocks get a -30000 triangle accumulated into the score
  PSUM by an identity matmul, so exp() yields exact zeros and the vector
  engine stays out of the score->attnv chain.
- emission is chunk-pipelined (QKV chunk c, attention chunk c, projection
  chunk c) so the PE always has dense matmul work while ScalarE runs exp.
"""

import sys

sys.path.insert(0, "/opt/trn_rl_repo")

import numpy as np

import concourse.bass as bass
import concourse.mybir as mybir
import concourse.tile as tile
from concourse import bacc
from concourse.bass_utils import run_bass_kernel_spmd

F32 = mybir.dt.float32
F16 = mybir.dt.float16
AFT = mybir.ActivationFunctionType

B, S, D, H, HD = 2, 2048, 1024, 16, 64
NCORES = 8
HPC = 4            # heads per core
CH = HPC * HD      # 256 channels per core
VW = HD + 1        # v width incl. ones column
P = 128
KT = D // P        # 8 contraction tiles over D
SQ = 512           # query/N chunk
NSQ = S // SQ      # 4
NST = S // P       # 16 seq tiles
SCALE = 1.0 / np.sqrt(HD)
MASKNEG = -30000.0


def emit_kernel(nc, tc, ap):
    """Emit the per-core program. `ap` is a dict of DRAM APs."""
    with (
        tc.tile_pool(name="const", bufs=1) as cp,
        tc.tile_pool(name="xw", bufs=1) as xw,
        tc.tile_pool(name="act", bufs=1) as acts,
        tc.tile_pool(name="ex", bufs=6) as exp_pool,
        tc.tile_pool(name="dh", bufs=4) as dh_pool,
        tc.tile_pool(name="rc", bufs=2) as rc_pool,
        tc.tile_pool(name="osb", bufs=3) as osb,
        tc.tile_pool(name="psA", bufs=2, space="PSUM") as psA,
        tc.tile_pool(name="psB", bufs=2, space="PSUM") as psB,
        tc.tile_pool(name="psC", bufs=2, space="PSUM") as psC,
    ):
        # ---- PE warmup: dense dummy matmuls while input DMAs stream in.
        # The PE clock-gate (HAM) unthrottles 1.2->2.4 GHz only after ~3.4us
        # of sustained matmul activity; burn that in on scratch data.
        wsrc = cp.tile([P, SQ], F16, name="wsrc", tag="wsrc")
        nc.gpsimd.memset(wsrc, 0.0)
        wps = psA.tile([P, SQ], F32, name="wps", tag="ps")
        for i in range(20):
            nc.tensor.matmul(
                wps, wsrc[:, 0:P], wsrc, start=(i == 0), stop=(i == 19),
            )

        # ---- constants ----
        ident = cp.tile([P, P], F16, name="ident", tag="ident")
        nc.sync.dma_start(ident, ap["ident"])
        mneg = cp.tile([P, P], F16, name="mneg", tag="mneg")
        nc.sync.dma_start(mneg, ap["mneg"])
        bq = cp.tile([P, 2], F32, name="bq", tag="bq")
        nc.sync.dma_start(bq, ap["bq"])
        bk = cp.tile([P, 2], F32, name="bk", tag="bk")
        nc.sync.dma_start(bk, ap["bk"])
        bv = cp.tile([1, HPC * VW], F16, name="bv", tag="bv")
        nc.sync.dma_start(bv, ap["bv"])
        ones1 = cp.tile([1, P], F16, name="ones1", tag="ones1")
        nc.sync.dma_start(ones1, ap["ones1"])

        # ---- weight/x loads (k-tile order so QKV can start early) ----
        xts, wq_t, wk_t, wv_t = [], [], [], []
        for k in range(KT):
            xt = xw.tile([P, S], F16, name=f"xt{k}", tag=f"xt{k}")
            nc.sync.dma_start(xt, ap["xT"][k * P:(k + 1) * P, :])
            xts.append(xt)
            w = xw.tile([P, CH], F16, name=f"wq{k}", tag=f"wq{k}")
            nc.sync.dma_start(w, ap["wq"][k * P:(k + 1) * P, :])
            wq_t.append(w)
            w = xw.tile([P, CH], F16, name=f"wk{k}", tag=f"wk{k}")
            nc.sync.dma_start(w, ap["wk"][k * P:(k + 1) * P, :])
            wk_t.append(w)
            w = xw.tile([P, HPC * VW], F16, name=f"wv{k}", tag=f"wv{k}")
            nc.sync.dma_start(w, ap["wv"][k * P:(k + 1) * P, :])
            wv_t.append(w)
        wp_t = []
        for k in range(2):
            w = xw.tile([P, D], F16, name=f"wp{k}", tag=f"wp{k}")
            nc.sync.dma_start(w, ap["wp"][k * P:(k + 1) * P, :])
            wp_t.append(w)

        # ---- activations living across phases ----
        qT = [acts.tile([P, S], F16, name=f"qT{i}", tag=f"qT{i}") for i in range(2)]
        kTt = [acts.tile([P, S], F16, name=f"kT{i}", tag=f"kT{i}") for i in range(2)]
        vv = acts.tile([P, NST, HPC * VW], F16, name="vv", tag="vv")
        outT = [acts.tile([P, S], F16, name=f"oT{i}", tag=f"oT{i}") for i in range(2)]

        def qkv_chunk(c):
            # qT/kT chunk c: [chan, seq] = w^T x^T : lhsT = w k-tile
            for dst, wt, bias in ((qT, wq_t, bq), (kTt, wk_t, bk)):
                for i in range(2):
                    ps = psA.tile([P, SQ], F32, name="ps", tag="ps")
                    for k in range(KT):
                        nc.tensor.matmul(
                            ps,
                            wt[k][:, i * P:(i + 1) * P],
                            xts[k][:, c * SQ:(c + 1) * SQ],
                            start=(k == 0),
                            stop=(k == KT - 1),
                        )
                    with nc.allow_low_precision(reason="fp16 matmul inputs"):
                        nc.vector.tensor_scalar_add(
                            dst[i][:, c * SQ:(c + 1) * SQ], ps, bias[:, i:i + 1],
                        )
            # v rows for this chunk (natural layout + interleaved ones cols)
            for st in range(4 * c, 4 * c + 4):
                ps = psA.tile([P, SQ], F32, name="psv", tag="ps")
                psv = ps[:, 0:HPC * VW]
                for k in range(KT):
                    nc.tensor.matmul(
                        psv,
                        xts[k][:, st * P:(st + 1) * P],
                        wv_t[k],
                        start=(k == 0),
                        stop=False,
                    )
                # += ones_col(seq) x (bv | interleaved 1.0): v-bias + ones col
                nc.tensor.matmul(psv, ones1, bv, start=False, stop=True)
                with nc.allow_low_precision(reason="fp16 matmul inputs"):
                    nc.vector.tensor_copy(vv[:, st, :], psv)

        def attention_head(h, c):
            ih, ro = h // 2, (h % 2) * 64
            nkt = 4 * (c + 1)
            kq = (kTt[ih][ro:ro + 64, :], qT[ih][ro:ro + 64, :])
            acc = psB.tile([VW, SQ], F32, name="acc", tag="acc")

            def score(dst_ps, kt, colo):
                diag = colo > 0 or kt * P == c * SQ
                nc.tensor.matmul(
                    dst_ps[:, colo:SQ],
                    kq[0][:, kt * P:(kt + 1) * P],
                    kq[1][:, c * SQ + colo:(c + 1) * SQ],
                    start=True,
                    stop=not diag,
                )
                if diag:  # diagonal block: add -30000 triangle before exp
                    nc.tensor.matmul(
                        dst_ps[:, colo:colo + P], ident, mneg,
                        start=False, stop=True,
                    )

            def attnv(ex_ap, kt, colo):
                nc.tensor.matmul(
                    acc[:, colo:SQ],
                    vv[:, kt, h * VW:(h + 1) * VW],
                    ex_ap,
                    start=(kt == 0),
                    stop=(kt == nkt - 1),
                )

            # full-width key tiles in pairs (one exp per pair)
            nfull = 4 * c
            for kt0 in range(0, nfull, 2):
                sc2 = psC.tile([P, 2, SQ], F32, name="sc2", tag="sc")
                score(sc2[:, 0], kt0, 0)
                score(sc2[:, 1], kt0 + 1, 0)
                ex2 = exp_pool.tile([P, 2, SQ], F16, name="ex2", tag="ex")
                nc.scalar.activation(ex2, sc2, AFT.Exp, scale=SCALE)
                attnv(ex2[:, 0, :], kt0, 0)
                attnv(ex2[:, 1, :], kt0 + 1, 0)
            # diagonal key tiles, restricted widths
            for kt in range(nfull, nkt):
                colo = kt * P - c * SQ
                sc1 = psC.tile([P, 2, SQ], F32, name="sc1", tag="sc")
                score(sc1[:, 0], kt, colo)
                ex1 = exp_pool.tile([P, 2, SQ], F16, name="ex1", tag="ex")
                nc.scalar.activation(
                    ex1[:, 0, colo:SQ], sc1[:, 0, colo:SQ], AFT.Exp, scale=SCALE,
                )
                attnv(ex1[:, 0, colo:SQ], kt, colo)

            with nc.allow_low_precision(reason="fp16 matmul inputs"):
                nc.vector.tensor_copy(
                    outT[ih][ro:ro + 64, c * SQ:(c + 1) * SQ], acc[0:64, :],
                )
            dn = dh_pool.tile([1, SQ], F16, name="dn", tag="dn")
            nc.scalar.activation(dn, acc[64:65, :], AFT.Copy)
            return dn

        def norm_chunk(c, denh):
            # outT *= 1/denominator: broadcast denoms via K=1 matmuls, one
            # 128-lane fast reciprocal, one fp16 multiply
            for i in range(2):
                db = psA.tile([P, SQ], F32, name="ps", tag="ps")
                nc.tensor.matmul(
                    db[0:64, :], ones1[:, 0:64], denh[2 * i],
                    start=True, stop=True,
                )
                nc.tensor.matmul(
                    db[64:P, :], ones1[:, 0:64], denh[2 * i + 1],
                    start=True, stop=True,
                )
                rc32 = rc_pool.tile([P, SQ], F32, name="rc32", tag="rc32")
                nc.vector.reciprocal_approx_fast(rc32, db)
                rcpb = rc_pool.tile([P, SQ], F16, name="rcpb", tag="rcpb")
                with nc.allow_low_precision(reason="fp16 matmul inputs"):
                    nc.vector.tensor_copy(rcpb, rc32)
                nc.vector.tensor_mul(
                    outT[i][:, c * SQ:(c + 1) * SQ],
                    outT[i][:, c * SQ:(c + 1) * SQ],
                    rcpb,
                )

        def proj_chunk(c):
            for m in range(4 * c, 4 * c + 4):
                for nch in range(2):
                    ps = psA.tile([P, SQ], F32, name="ps", tag="ps")
                    for kk in range(2):
                        nc.tensor.matmul(
                            ps,
                            outT[kk][:, m * P:(m + 1) * P],
                            wp_t[kk][:, nch * SQ:(nch + 1) * SQ],
                            start=(kk == 0),
                            stop=(kk == 1),
                        )
                    ob = osb.tile([P, SQ], F32, name="ob", tag="ob")
                    nc.vector.tensor_copy(ob, ps)
                    nc.sync.dma_start(
                        ap["out"][m * P:(m + 1) * P, nch * SQ:(nch + 1) * SQ], ob,
                    )

        # ---- chunk-pipelined main body ----
        for c in range(NSQ):
            qkv_chunk(c)
            denh = [attention_head(h, c) for h in range(HPC)]
            norm_chunk(c, denh)
            proj_chunk(c)


def build_program():
    nc = bacc.Bacc("TRN2", target_bir_lowering=False, debug=False,
                   num_devices=NCORES)
    ap = {}
    for name, shape, dt in (
        ("xT", [D, S], F16), ("wq", [D, CH], F16), ("wk", [D, CH], F16),
        ("wv", [D, HPC * VW], F16), ("bq", [P, 2], F32), ("bk", [P, 2], F32),
        ("bv", [1, HPC * VW], F16), ("wp", [CH, D], F16),
        ("ident", [P, P], F16), ("mneg", [P, P], F16), ("ones1", [1, P], F16),
    ):
        ap[name] = nc.dram_tensor(name, shape, dt, kind="ExternalInput").ap()
    ap["out"] = nc.dram_tensor("out", [S, D], F32, kind="ExternalOutput").ap()

    with tile.TileContext(nc) as tc:
        emit_kernel(nc, tc, ap)
    nc.compile()
    return nc


def make_core_inputs(hidden_states, w_attn, b_attn, w_proj):
    """Host-side sharding: per-core input dicts (core = batch*4 + head_group)."""
    f16, f32 = np.float16, np.float32
    x = np.asarray(hidden_states, f32)
    w_attn = np.asarray(w_attn, f32)
    b_attn = np.asarray(b_attn, f32)
    w_proj = np.asarray(w_proj, f32)

    ident = np.eye(P, dtype=f16)
    mneg = np.where(np.arange(P)[:, None] <= np.arange(P)[None, :],
                    0.0, MASKNEG).astype(f16)
    ones_row = np.ones((1, P), f16)
    xTs = [np.ascontiguousarray(x[b].T).astype(f16) for b in range(B)]

    in_maps = []
    for core in range(NCORES):
        b, g = core // HPC, core % HPC
        wq = np.ascontiguousarray(w_attn[:, g * CH:(g + 1) * CH]).astype(f16)
        wk = np.ascontiguousarray(
            w_attn[:, D + g * CH:D + (g + 1) * CH]).astype(f16)
        wv = np.zeros((D, HPC * VW), f16)
        bv = np.zeros((1, HPC * VW), f16)
        for h in range(HPC):
            src = 2 * D + (g * HPC + h) * HD
            wv[:, h * VW:h * VW + HD] = w_attn[:, src:src + HD]
            bv[0, h * VW:h * VW + HD] = b_attn[src:src + HD]
            bv[0, h * VW + HD] = 1.0
        bq = np.ascontiguousarray(
            b_attn[g * CH:(g + 1) * CH].reshape(2, P).T)
        bk = np.ascontiguousarray(
            b_attn[D + g * CH:D + (g + 1) * CH].reshape(2, P).T)
        wp = np.ascontiguousarray(w_proj[g * CH:(g + 1) * CH, :]).astype(f16)
        in_maps.append({
            "xT": xTs[b], "wq": wq, "wk": wk, "wv": wv,
            "bq": bq, "bk": bk, "bv": bv, "wp": wp,
            "ident": ident, "mneg": mneg, "ones1": ones_row,
        })
    return in_maps


_PROGRAM = None


def kernel(hidden_states, w_attn, b_attn, w_proj, b_proj):
    global _PROGRAM
    if _PROGRAM is None:
        _PROGRAM = build_program()
    in_maps = make_core_inputs(hidden_states, w_attn, b_attn, w_proj)
    res = run_bass_kernel_spmd(_PROGRAM, in_maps, core_ids=list(range(NCORES)))
    out = np.zeros((B, S, D), np.float32)
    for core in range(NCORES):
        out[core // HPC] += res.results[core]["out"]
    out += np.asarray(b_proj, np.float32)
    return out


# revision 17
# speedup vs baseline: 1.3499x; 1.3499x over previous
"""Fused causal multi-head attention block on 8 Trainium2 NeuronCores.

Problem (GPT-2 style attention, B=2, S=2048, D=1024, H=16, hd=64):
    qkv = x @ w_attn + b_attn ; split q,k,v ; per-head causal softmax(q k^T / 8) v
    out = attn_out @ w_proj + b_proj

Sharding: data parallel on batch (2) x tensor parallel on heads (4 groups of 4
heads). Core c -> batch c//4, head group c%4. Each core computes a partial
[S, D] output (its heads' slice of w_proj rows); host sums the 4 partials per
batch and adds b_proj.

Per-core kernel layout tricks:
- scores are computed TRANSPOSED (scoresT[key, query]) so the softmax
  denominator falls out of the attn@v matmul by appending a ones-column to v:
  [v | 1]^T @ exp(scoresT) yields the unnormalized output and the per-query
  denominator in one PSUM accumulation.
- matmul inputs are fp16 (full PE rate + fast weight loads); all accumulation
  is fp32 in PSUM. exp(s/8) is in [0, ~13], well inside fp16 range.
- causal masking: fully-masked blocks are skipped via restricted matmul
  widths; diagonal blocks get a -30000 triangle accumulated into the score
  PSUM by an identity matmul, so exp() yields exact zeros and the vector
  engine stays out of the score->attnv chain.
- emission is chunk-pipelined (QKV chunk c, attention chunk c, projection
  chunk c) so the PE always has dense matmul work while ScalarE runs exp.
"""

import sys

sys.path.insert(0, "/opt/trn_rl_repo")

import numpy as np

import concourse.bass as bass
import concourse.mybir as mybir
import concourse.tile as tile
from concourse import bacc
from concourse.bass_utils import run_bass_kernel_spmd

F32 = mybir.dt.float32
F16 = mybir.dt.float16
AFT = mybir.ActivationFunctionType

B, S, D, H, HD = 2, 2048, 1024, 16, 64
NCORES = 8
HPC = 4            # heads per core
CH = HPC * HD      # 256 channels per core
VW = HD + 1        # v width incl. ones column
P = 128
KT = D // P        # 8 contraction tiles over D
SQ = 512           # query/N chunk
NSQ = S // SQ      # 4
NST = S // P       # 16 seq tiles
SCALE = 1.0 / np.sqrt(HD)
MASKNEG = -30000.0


def emit_kernel(nc, tc, ap):
    """Emit the per-core program. `ap` is a dict of DRAM APs."""
    with (
        tc.tile_pool(name="const", bufs=1) as cp,
        tc.tile_pool(name="xw", bufs=1) as xw,
        tc.tile_pool(name="act", bufs=1) as acts,
        tc.tile_pool(name="ex", bufs=10) as exp_pool,
        tc.tile_pool(name="dh", bufs=4) as dh_pool,
        tc.tile_pool(name="rc", bufs=2) as rc_pool,
        tc.tile_pool(name="osb", bufs=3) as osb,
        tc.tile_pool(name="psA", bufs=3, space="PSUM") as psA,
        tc.tile_pool(name="psB", bufs=2, space="PSUM") as psB,
        tc.tile_pool(name="psC", bufs=3, space="PSUM") as psC,
    ):
        # ---- PE warmup: dense dummy matmuls while input DMAs stream in.
        # The PE clock-gate (HAM) unthrottles 1.2->2.4 GHz only after ~3.4us
        # of sustained matmul activity; burn that in on scratch data.
        wsrc = cp.tile([P, SQ], F16, name="wsrc", tag="wsrc")
        nc.gpsimd.memset(wsrc, 0.0)
        wps = psA.tile([P, SQ], F32, name="wps", tag="ps")
        for i in range(20):
            nc.tensor.matmul(
                wps, wsrc[:, 0:P], wsrc, start=(i == 0), stop=(i == 19),
            )

        # ---- constants ----
        ident = cp.tile([P, P], F16, name="ident", tag="ident")
        nc.sync.dma_start(ident, ap["ident"])
        mneg = cp.tile([P, P], F16, name="mneg", tag="mneg")
        nc.sync.dma_start(mneg, ap["mneg"])
        bq = cp.tile([P, 2], F32, name="bq", tag="bq")
        nc.sync.dma_start(bq, ap["bq"])
        bk = cp.tile([P, 2], F32, name="bk", tag="bk")
        nc.sync.dma_start(bk, ap["bk"])
        bv = cp.tile([1, HPC * VW], F16, name="bv", tag="bv")
        nc.sync.dma_start(bv, ap["bv"])
        ones1 = cp.tile([1, P], F16, name="ones1", tag="ones1")
        nc.sync.dma_start(ones1, ap["ones1"])

        # ---- weight/x loads (k-tile order so QKV can start early) ----
        xts, wq_t, wk_t, wv_t = [], [], [], []
        for k in range(KT):
            xt = xw.tile([P, S], F16, name=f"xt{k}", tag=f"xt{k}")
            nc.sync.dma_start(xt, ap["xT"][k * P:(k + 1) * P, :])
            xts.append(xt)
            w = xw.tile([P, CH], F16, name=f"wq{k}", tag=f"wq{k}")
            nc.sync.dma_start(w, ap["wq"][k * P:(k + 1) * P, :])
            wq_t.append(w)
            w = xw.tile([P, CH], F16, name=f"wk{k}", tag=f"wk{k}")
            nc.sync.dma_start(w, ap["wk"][k * P:(k + 1) * P, :])
            wk_t.append(w)
            w = xw.tile([P, HPC * VW], F16, name=f"wv{k}", tag=f"wv{k}")
            nc.sync.dma_start(w, ap["wv"][k * P:(k + 1) * P, :])
            wv_t.append(w)
        wp_t = []
        for k in range(2):
            w = xw.tile([P, D], F16, name=f"wp{k}", tag=f"wp{k}")
            nc.sync.dma_start(w, ap["wp"][k * P:(k + 1) * P, :])
            wp_t.append(w)

        # ---- activations living across phases ----
        qT = [acts.tile([P, S], F16, name=f"qT{i}", tag=f"qT{i}") for i in range(2)]
        kTt = [acts.tile([P, S], F16, name=f"kT{i}", tag=f"kT{i}") for i in range(2)]
        vv = acts.tile([P, NST, HPC * VW], F16, name="vv", tag="vv")
        outT = [acts.tile([P, S], F16, name=f"oT{i}", tag=f"oT{i}") for i in range(2)]

        def qkv_chunk(c):
            # qT/kT chunk c: [chan, seq] = w^T x^T : lhsT = w k-tile
            for dst, wt, bias in ((qT, wq_t, bq), (kTt, wk_t, bk)):
                for i in range(2):
                    ps = psA.tile([P, SQ], F32, name="ps", tag="ps")
                    for k in range(KT):
                        nc.tensor.matmul(
                            ps,
                            wt[k][:, i * P:(i + 1) * P],
                            xts[k][:, c * SQ:(c + 1) * SQ],
                            start=(k == 0),
                            stop=(k == KT - 1),
                        )
                    with nc.allow_low_precision(reason="fp16 matmul inputs"):
                        nc.vector.tensor_scalar_add(
                            dst[i][:, c * SQ:(c + 1) * SQ], ps, bias[:, i:i + 1],
                        )
            # v rows for this chunk (natural layout + interleaved ones cols)
            for st in range(4 * c, 4 * c + 4):
                ps = psA.tile([P, SQ], F32, name="psv", tag="ps")
                psv = ps[:, 0:HPC * VW]
                for k in range(KT):
                    nc.tensor.matmul(
                        psv,
                        xts[k][:, st * P:(st + 1) * P],
                        wv_t[k],
                        start=(k == 0),
                        stop=False,
                    )
                # += ones_col(seq) x (bv | interleaved 1.0): v-bias + ones col
                nc.tensor.matmul(psv, ones1, bv, start=False, stop=True)
                with nc.allow_low_precision(reason="fp16 matmul inputs"):
                    nc.vector.tensor_copy(vv[:, st, :], psv)

        def attention_head(h, c):
            ih, ro = h // 2, (h % 2) * 64
            nkt = 4 * (c + 1)
            kq = (kTt[ih][ro:ro + 64, :], qT[ih][ro:ro + 64, :])
            acc = psB.tile([VW, SQ], F32, name="acc", tag="acc")

            def score(dst_ps, kt, colo):
                diag = colo > 0 or kt * P == c * SQ
                nc.tensor.matmul(
                    dst_ps[:, colo:SQ],
                    kq[0][:, kt * P:(kt + 1) * P],
                    kq[1][:, c * SQ + colo:(c + 1) * SQ],
                    start=True,
                    stop=not diag,
                )
                if diag:  # diagonal block: add -30000 triangle before exp
                    nc.tensor.matmul(
                        dst_ps[:, colo:colo + P], ident, mneg,
                        start=False, stop=True,
                    )

            def attnv(ex_ap, kt, colo):
                nc.tensor.matmul(
                    acc[:, colo:SQ],
                    vv[:, kt, h * VW:(h + 1) * VW],
                    ex_ap,
                    start=(kt == 0),
                    stop=(kt == nkt - 1),
                )

            # emit ALL scores (+exp) first, then all attnv matmuls: the PE
            # stream is in-order, so this keeps the PE busy on scores while
            # ScalarE's exps pipeline behind, instead of stalling on each exp
            exs = []
            for kt in range(nkt):
                colo = max(0, kt * P - c * SQ)
                sc1 = psC.tile([P, SQ], F32, name="sc1", tag="sc")
                score(sc1, kt, colo)
                ex1 = exp_pool.tile([P, SQ], F16, name="ex1", tag="ex")
                nc.scalar.activation(
                    ex1[:, colo:SQ], sc1[:, colo:SQ], AFT.Exp, scale=SCALE,
                )
                exs.append((ex1, kt, colo))
            for ex1, kt, colo in exs:
                attnv(ex1[:, colo:SQ], kt, colo)

            with nc.allow_low_precision(reason="fp16 matmul inputs"):
                nc.vector.tensor_copy(
                    outT[ih][ro:ro + 64, c * SQ:(c + 1) * SQ], acc[0:64, :],
                )
            dn = dh_pool.tile([1, SQ], F16, name="dn", tag="dn")
            nc.scalar.activation(dn, acc[64:65, :], AFT.Copy)
            return dn

        def norm_chunk(c, denh):
            # outT *= 1/denominator: broadcast denoms via K=1 matmuls, one
            # 128-lane fast reciprocal, one fp16 multiply
            for i in range(2):
                db = psA.tile([P, SQ], F32, name="ps", tag="ps")
                nc.tensor.matmul(
                    db[0:64, :], ones1[:, 0:64], denh[2 * i],
                    start=True, stop=True,
                )
                nc.tensor.matmul(
                    db[64:P, :], ones1[:, 0:64], denh[2 * i + 1],
                    start=True, stop=True,
                )
                rc32 = rc_pool.tile([P, SQ], F32, name="rc32", tag="rc32")
                nc.vector.reciprocal_approx_fast(rc32, db)
                rcpb = rc_pool.tile([P, SQ], F16, name="rcpb", tag="rcpb")
                with nc.allow_low_precision(reason="fp16 matmul inputs"):
                    nc.vector.tensor_copy(rcpb, rc32)
                nc.vector.tensor_mul(
                    outT[i][:, c * SQ:(c + 1) * SQ],
                    outT[i][:, c * SQ:(c + 1) * SQ],
                    rcpb,
                )

        def proj_mtile(m):
            for nch in range(2):
                ps = psA.tile([P, SQ], F32, name="ps", tag="ps")
                for kk in range(2):
                    nc.tensor.matmul(
                        ps,
                        outT[kk][:, m * P:(m + 1) * P],
                        wp_t[kk][:, nch * SQ:(nch + 1) * SQ],
                        start=(kk == 0),
                        stop=(kk == 1),
                    )
                ob = osb.tile([P, SQ], F32, name="ob", tag="ob")
                nc.vector.tensor_copy(ob, ps)
                nc.sync.dma_start(
                    ap["out"][m * P:(m + 1) * P, nch * SQ:(nch + 1) * SQ], ob,
                )

        # ---- chunk-pipelined main body: projection of chunk c-1 is
        # interleaved between attention heads of chunk c to keep the PE fed
        # while ScalarE runs the exps ----
        for c in range(NSQ):
            qkv_chunk(c)
            denh = []
            for h in range(HPC):
                denh.append(attention_head(h, c))
                if c > 0:
                    proj_mtile(4 * (c - 1) + h)
            norm_chunk(c, denh)
        for m in range(4 * (NSQ - 1), 4 * NSQ):
            proj_mtile(m)


def build_program():
    nc = bacc.Bacc("TRN2", target_bir_lowering=False, debug=False,
                   num_devices=NCORES)
    ap = {}
    for name, shape, dt in (
        ("xT", [D, S], F16), ("wq", [D, CH], F16), ("wk", [D, CH], F16),
        ("wv", [D, HPC * VW], F16), ("bq", [P, 2], F32), ("bk", [P, 2], F32),
        ("bv", [1, HPC * VW], F16), ("wp", [CH, D], F16),
        ("ident", [P, P], F16), ("mneg", [P, P], F16), ("ones1", [1, P], F16),
    ):
        ap[name] = nc.dram_tensor(name, shape, dt, kind="ExternalInput").ap()
    ap["out"] = nc.dram_tensor("out", [S, D], F32, kind="ExternalOutput").ap()

    with tile.TileContext(nc) as tc:
        emit_kernel(nc, tc, ap)
    nc.compile()
    return nc


def make_core_inputs(hidden_states, w_attn, b_attn, w_proj):
    """Host-side sharding: per-core input dicts (core = batch*4 + head_group)."""
    f16, f32 = np.float16, np.float32
    x = np.asarray(hidden_states, f32)
    w_attn = np.asarray(w_attn, f32)
    b_attn = np.asarray(b_attn, f32)
    w_proj = np.asarray(w_proj, f32)

    ident = np.eye(P, dtype=f16)
    mneg = np.where(np.arange(P)[:, None] <= np.arange(P)[None, :],
                    0.0, MASKNEG).astype(f16)
    ones_row = np.ones((1, P), f16)
    xTs = [np.ascontiguousarray(x[b].T).astype(f16) for b in range(B)]

    in_maps = []
    for core in range(NCORES):
        b, g = core // HPC, core % HPC
        wq = np.ascontiguousarray(w_attn[:, g * CH:(g + 1) * CH]).astype(f16)
        wk = np.ascontiguousarray(
            w_attn[:, D + g * CH:D + (g + 1) * CH]).astype(f16)
        wv = np.zeros((D, HPC * VW), f16)
        bv = np.zeros((1, HPC * VW), f16)
        for h in range(HPC):
            src = 2 * D + (g * HPC + h) * HD
            wv[:, h * VW:h * VW + HD] = w_attn[:, src:src + HD]
            bv[0, h * VW:h * VW + HD] = b_attn[src:src + HD]
            bv[0, h * VW + HD] = 1.0
        bq = np.ascontiguousarray(
            b_attn[g * CH:(g + 1) * CH].reshape(2, P).T)
        bk = np.ascontiguousarray(
            b_attn[D + g * CH:D + (g + 1) * CH].reshape(2, P).T)
        wp = np.ascontiguousarray(w_proj[g * CH:(g + 1) * CH, :]).astype(f16)
        in_maps.append({
            "xT": xTs[b], "wq": wq, "wk": wk, "wv": wv,
            "bq": bq, "bk": bk, "bv": bv, "wp": wp,
            "ident": ident, "mneg": mneg, "ones1": ones_row,
        })
    return in_maps


_PROGRAM = None


def kernel(hidden_states, w_attn, b_attn, w_proj, b_proj):
    global _PROGRAM
    if _PROGRAM is None:
        _PROGRAM = build_program()
    in_maps = make_core_inputs(hidden_states, w_attn, b_attn, w_proj)
    res = run_bass_kernel_spmd(_PROGRAM, in_maps, core_ids=list(range(NCORES)))
    out = np.zeros((B, S, D), np.float32)
    for core in range(NCORES):
        out[core // HPC] += res.results[core]["out"]
    out += np.asarray(b_proj, np.float32)
    return out
